# revision 38
# baseline (speedup 1.0000x reference)
"""Trainium2 Bass kernel for nn_LSTMAutoencoder (B=512, T=256, D=H=128).

Strategy: 8-way data-parallel over batch (64/core). On-chip layout keeps
H on partitions and batch on the free dim so the recurrence needs no
transposes. Gate order is repacked host-side to [f, i, o, 2g] so one
sigmoid activation op covers all four gates (tanh(g) = 2*sigmoid(2g)-1,
recovered for free inside a fused scalar_tensor_tensor op). Encoder
layers 0/1 run as a fused wavefront (both cells share one PSUM bank,
one sigmoid op, and paired DVE ops). All weights are pre-transposed,
fp16, with biases applied via a tiny K=4/8 indicator matmul into PSUM.

Execution path: the axon tunnel moves ~40-45MB/s shared, so e2e wall
time is transfer-dominated (device exec itself is ~10ms; a full
exec+dispatch round trip is ~70ms). A custom PJRT runner (replacing
run_bass_kernel_spmd, which re-traces and re-uploads everything
including 67MB of zero-filled output buffers every call) caches the
jitted executables across calls and keeps all weight tensors
device-resident. Wire format is minimal: x goes up as fp8-e4m3
(16.8MB, t-major, upcast to fp16 on-chip; adds <2e-4 to the final
error), y comes back as int8 with a per-batch-row dynamic scale
(16.8MB; the store converts with RNE so quantization adds <=0.5
step = ~4e-3 worst case). y is built transposed on-chip (out-projection
runs as h^T @ W^T with lhsT=h) so both the device DMA and the host
dequant are contiguous. The batch is split across two 4-core meshes
whose upload/exec/download pipelines overlap via worker threads, with
the GIL-bound fp8 host conversion kept sequential on the main thread,
interleaved with async uploads. Steady state: ~0.9s/call vs 6.0s for
the baseline runner (vs ~0.78s of raw wire time for 34MB).
"""

import os
import sys
import time
import numpy as np

sys.path.insert(0, '/opt/trn_rl_repo')

B, T_FULL, D, H = 512, 256, 128, 128
NCORES = 8
BL = B // NCORES  # 64 batch per core

_cache = {}


def _f16(a):
    return np.ascontiguousarray(a).astype(np.float16)


def _prep_layer(Wih, Whh, bih, bhh, x_is_h):
    # torch gate order i,f,g,o -> [f, i, o, 2g]; transpose for lhsT use.
    # States on-chip are H2=2h, so any weight column that consumes h is
    # pre-halved (all Whh; Wih too when the layer input is a hidden state).
    def re(M):
        i, f, g, o = M[0:H], M[H:2*H], M[2*H:3*H], M[3*H:4*H]
        return np.concatenate([f, i, o, 2.0 * g], 0)
    wih = re(Wih) * (0.5 if x_is_h else 1.0)
    wt = np.concatenate([wih.T, 0.5 * re(Whh).T], 1)    # [Din, 1024]
    bs = re((bih + bhh)[:, None])[:, 0].reshape(4, H)   # [4,128]
    return _f16(wt), _f16(bs)


def _build(T):
    import concourse.bass as bass  # noqa: F401
    import concourse.tile as tile
    from concourse import bacc, mybir
    from contextlib import ExitStack

    f16, f32 = mybir.dt.float16, mybir.dt.float32
    f8 = mybir.dt.float8e4
    AO = mybir.AluOpType
    AF = mybir.ActivationFunctionType

    nc = bacc.Bacc("TRN2", target_bir_lowering=False, debug=False,
                   enable_asserts=False, num_devices=NCORES)

    def din(name, shape, dt=f16):
        return nc.dram_tensor(name, shape, dt, kind="ExternalInput").ap()

    xT = din('xT', [128, T * BL], f8)
    wts = {L: din('wt_' + L, [128, 1024]) for L in ('e0', 'e1', 'd0', 'd1')}
    bse8 = din('bse8', [8, 128])
    bss = {L: din('bs_' + L, [4, 128]) for L in ('e0', 'e1', 'd0', 'd1')}
    ind8 = din('ind8', [8, 8 * BL])
    ind4 = din('ind4', [4, 4 * BL])
    outw = din('outw', [128, 128])
    outb = din('outb', [1, 128])
    ones = din('ones', [1, BL])
    # y stored [b, t*128+d] per core as int8 with a per-row dynamic scale:
    # contiguous DMA runs on-device, transpose-free dequant on host, and
    # half the download bytes of f16. int8 convert is RNE on HW, so the
    # quantization error is <= 0.5 * rowmax/127 ~ 4e-3 of absmax worst case.
    i8 = mybir.dt.int8
    yQ = nc.dram_tensor('yQ', [BL, T * 128], i8, kind="ExternalOutput").ap()
    ysc = nc.dram_tensor('ysc', [BL, 1], f32, kind="ExternalOutput").ap()

    with tile.TileContext(nc) as tc, ExitStack() as ctx:
        cst = ctx.enter_context(tc.tile_pool(name="cst", bufs=1))
        gp = ctx.enter_context(tc.tile_pool(name="gp", bufs=4, space="PSUM"))
        yp = ctx.enter_context(tc.tile_pool(name="ypp", bufs=3, space="PSUM"))
        sb = ctx.enter_context(tc.tile_pool(name="sb", bufs=4))
        st = ctx.enter_context(tc.tile_pool(name="st", bufs=4))
        yf = ctx.enter_context(tc.tile_pool(name="yf", bufs=1))
        qp = ctx.enter_context(tc.tile_pool(name="qp", bufs=1))

        # ---- load constants into SBUF
        def cload(ap, shape, tag):
            t = cst.tile(shape, f16, tag=tag)
            nc.sync.dma_start(t[:], ap)
            return t

        xsb8 = cst.tile([128, T * BL], f8, tag='xsb8')
        nc.sync.dma_start(xsb8[:], xT)
        xsb = cst.tile([128, T * BL], f16, tag='xsb')
        nc.scalar.copy(xsb[:], xsb8[:])  # one-shot fp8 -> fp16 upcast
        wsb = {L: cload(wts[L], [128, 1024], 'w' + L) for L in wts}
        bse8s = cload(bse8, [8, 128], 'bse8')
        bsbs = {L: cload(bss[L], [4, 128], 'bs' + L) for L in bss}
        ind8s = cload(ind8, [8, 8 * BL], 'ind8')
        ind4s = cload(ind4, [4, 4 * BL], 'ind4')
        outws = cload(outw, [128, 128], 'outw')
        outbs = cload(outb, [1, 128], 'outb')
        oness = cload(ones, [1, BL], 'ones')

        MM = nc.tensor.matmul
        STT = nc.vector.scalar_tensor_tensor

        # single LSTM cell: [128, BL] tiles, gates psum [128, 4*BL]
        def cell(wt, bs, x_ap, h_ap, c_ap, hout_ap, cout_ap, skip_hh, sfx):
            g = gp.tile([128, 4 * BL], f32, tag='g')
            # hh matmuls first: their input is ready one cell earlier, so
            # the PE runs them while the previous cell's elementwise tail
            # is still in flight; only ih-MMs + bias sit on the chain.
            if not skip_hh:
                for k in range(4):
                    MM(g[:, k*BL:(k+1)*BL], wt[:, 512+k*128:512+(k+1)*128],
                       h_ap, start=True, stop=False)
            for k in range(4):
                MM(g[:, k*BL:(k+1)*BL], wt[:, k*128:(k+1)*128], x_ap,
                   start=skip_hh, stop=False)
            MM(g[:, :], bs[:4, :], ind4s[:4, :], start=False, stop=True)
            s = sb.tile([128, 4 * BL], f16, tag='s')
            nc.scalar.activation(s[:], g[:], AF.Tanh, scale=0.5)
            tf, ti, to_, tg = (s[:, 0:BL], s[:, BL:2*BL],
                               s[:, 2*BL:3*BL], s[:, 3*BL:4*BL])
            u = sb.tile([128, BL], f16, tag='u')
            STT(u[:], ti, 1.0, tg, AO.add, AO.mult)       # 2*sig(i)*tanh(g)
            X = sb.tile([128, BL], f32, tag='X')
            STT(X[:], tf, 1.0, c_ap, AO.add, AO.mult)     # 2*sig(f)*C2
            STT(cout_ap, X[:], 0.5, u[:], AO.mult, AO.add)  # C2' = 2c'
            th = sb.tile([128, BL], f16, tag='th')
            nc.scalar.activation(th[:], cout_ap, AF.Tanh, scale=0.5)
            STT(hout_ap, to_, 1.0, th[:], AO.add, AO.mult)  # H2 = 2h

        # fused encoder superstep: cell0=enc0(t), cell1=enc1(t-1)
        # psum layout [128, 8*BL]: block (k, c) at (2k+c)*BL
        def fused(t, eh_prev, ec_prev, eh_new, ec_new):
            g = gp.tile([128, 8 * BL], f32, tag='g')
            x_ap = xsb[:, t*BL:(t+1)*BL]
            h0 = eh_prev[:, 0:BL]
            h1 = eh_prev[:, BL:2*BL]
            for k in range(4):
                MM(g[:, (2*k)*BL:(2*k+1)*BL],
                   wsb['e0'][:, 512+k*128:512+(k+1)*128], h0,
                   start=True, stop=False)
                MM(g[:, (2*k+1)*BL:(2*k+2)*BL],
                   wsb['e1'][:, 512+k*128:512+(k+1)*128], h1,
                   start=True, stop=False)
            for k in range(4):
                MM(g[:, (2*k)*BL:(2*k+1)*BL], wsb['e0'][:, k*128:(k+1)*128],
                   x_ap, start=False, stop=False)
                MM(g[:, (2*k+1)*BL:(2*k+2)*BL], wsb['e1'][:, k*128:(k+1)*128],
                   h0, start=False, stop=False)
            MM(g[:, :], bse8s[:8, :], ind8s[:8, :], start=False, stop=True)
            s = sb.tile([128, 8 * BL], f16, tag='s')
            nc.scalar.activation(s[:], g[:], AF.Tanh, scale=0.5)
            P = 2 * BL
            tf, ti, to_, tg = (s[:, 0:P], s[:, P:2*P],
                               s[:, 2*P:3*P], s[:, 3*P:4*P])
            u = sb.tile([128, P], f16, tag='u')
            STT(u[:], ti, 1.0, tg, AO.add, AO.mult)
            X = sb.tile([128, P], f32, tag='X')
            STT(X[:], tf, 1.0, ec_prev[:], AO.add, AO.mult)
            STT(ec_new[:], X[:], 0.5, u[:], AO.mult, AO.add)
            th = sb.tile([128, P], f16, tag='th')
            nc.scalar.activation(th[:], ec_new[:], AF.Tanh, scale=0.5)
            STT(eh_new[:], to_, 1.0, th[:], AO.add, AO.mult)

        # ---- encoder
        eh = st.tile([128, 2 * BL], f16, tag='eh')
        ec = st.tile([128, 2 * BL], f32, tag='ec')
        nc.vector.memset(eh[:], 0.0)
        nc.vector.memset(ec[:], 0.0)

        # t=0: enc0 only (h,c zero; skip hh)
        eh_n = st.tile([128, 2 * BL], f16, tag='eh')
        ec_n = st.tile([128, 2 * BL], f32, tag='ec')
        nc.vector.memset(eh_n[:], 0.0)
        nc.vector.memset(ec_n[:], 0.0)
        cell(wsb['e0'], bsbs['e0'], xsb[:, 0:BL], None, ec[:, 0:BL],
             eh_n[:, 0:BL], ec_n[:, 0:BL], True, 'e0z')
        eh, ec = eh_n, ec_n

        for t in range(1, T):
            eh_n = st.tile([128, 2 * BL], f16, tag='eh')
            ec_n = st.tile([128, 2 * BL], f32, tag='ec')
            fused(t, eh, ec, eh_n, ec_n)
            eh, ec = eh_n, ec_n

        # tail: enc1 consumes h0(T-1)
        h1f = st.tile([128, BL], f16, tag='h1f')
        c1f = st.tile([128, BL], f32, tag='c1f')
        cell(wsb['e1'], bsbs['e1'], eh[:, 0:BL], eh[:, BL:2*BL],
             ec[:, BL:2*BL], h1f[:], c1f[:], False, 'e1z')

        # ---- decoder
        hx = h1f
        hd0 = st.tile([128, BL], f16, tag='hd0')
        cd0 = st.tile([128, BL], f32, tag='cd0')
        hd1 = st.tile([128, BL], f16, tag='hd1')
        cd1 = st.tile([128, BL], f32, tag='cd1')
        for z in (hd0, cd0, hd1, cd1):
            nc.vector.memset(z[:], 0.0)

        ysb = yf.tile([BL, T * 128], f16, tag='ysb')
        for t in range(T):
            hd0n = st.tile([128, BL], f16, tag='hd0')
            cd0n = st.tile([128, BL], f32, tag='cd0')
            cell(wsb['d0'], bsbs['d0'], hx[:], hd0[:], cd0[:],
                 hd0n[:], cd0n[:], t == 0, 'd0')
            hd1n = st.tile([128, BL], f16, tag='hd1')
            cd1n = st.tile([128, BL], f32, tag='cd1')
            cell(wsb['d1'], bsbs['d1'], hd0n[:], hd1[:], cd1[:],
                 hd1n[:], cd1n[:], t == 0, 'd1')
            hd0, cd0, hd1, cd1 = hd0n, cd0n, hd1n, cd1n
            # transposed out-projection: y^T[b,d] = (h^T @ W_out^T)[b,d]
            # (lhsT = hd1 directly; outws already holds 0.5*W_out^T)
            y = yp.tile([BL, 128], f32, tag='yp')
            MM(y[:], hd1[:], outws[:], start=True, stop=False)
            MM(y[:], oness[:1, :], outbs[:1, :], start=False, stop=True)
            nc.scalar.copy(ysb[:, t*128:(t+1)*128], y[:])
            hx = hd1

        # ---- dynamic per-row int8 quantization of y
        red = sb.tile([BL, 1], f32, tag='red')
        nc.vector.tensor_reduce(red[:], ysb[:], mybir.AxisListType.X,
                                AO.max, apply_absolute_value=True)
        ysch = sb.tile([BL, 1], f32, tag='ysch')
        nc.vector.tensor_scalar_mul(ysch[:], red[:], 1.0 / 127.0)
        nc.sync.dma_start(ysc, ysch[:])
        sc = sb.tile([BL, 1], f32, tag='sc')
        nc.vector.reciprocal(sc[:], ysch[:])
        q = qp.tile([BL, T * 128], i8, tag='q')
        nc.scalar.activation(q[:], ysb[:], AF.Copy, scale=sc[:])
        nc.sync.dma_start(yQ, q[:])

    nc.compile()
    return nc


class _Runner:
    """Cached PJRT execution of a compiled Bass module on a group of axon
    cores.

    Replaces run_bass_kernel_spmd's per-call closure (which re-traces,
    re-uploads every input, and uploads zero-filled output buffers each
    call). Constants live on device; only x moves per call. Running two
    runners on disjoint 4-core meshes from two threads pipelines the
    upload/exec/download of one batch half against the other.
    """

    def __init__(self, nc, devices):
        import jax
        import jax.numpy as jnp
        from jax.sharding import Mesh, PartitionSpec, NamedSharding
        from jax.experimental.shard_map import shard_map
        from concourse import bass2jax, mybir

        bass2jax.install_neuronx_cc_hook()
        self.jax = jax
        self.nc = nc
        self.ndev = len(devices)

        partition_name = (nc.partition_id_tensor.name
                          if nc.partition_id_tensor else None)
        dbg_name = nc.dbg_addr.name if nc.dbg_addr is not None else None
        if nc.dbg_addr is not None and nc.dbg_callbacks:
            raise RuntimeError('dbg_callbacks unsupported on axon client')

        in_names, out_names, out_avals = [], [], []
        for alloc in nc.m.functions[0].allocations:
            if not isinstance(alloc, mybir.MemoryLocationSet):
                continue
            name = alloc.memorylocations[0].name
            if alloc.kind == 'ExternalInput':
                if name != partition_name:
                    in_names.append(name)
            elif alloc.kind == 'ExternalOutput':
                out_names.append(name)
                out_avals.append(jax.core.ShapedArray(
                    tuple(alloc.tensor_shape), mybir.dt.np(alloc.dtype)))
        self.in_names = in_names
        self.out_names = out_names
        self.dbg_name = dbg_name

        bind_in_names = tuple(in_names) + tuple(out_names)
        if partition_name is not None:
            bind_in_names += (partition_name,)

        def _body(*args):
            # args = real inputs + placeholder output operands. The hook
            # renames each NEFF output tensor to output{i} only, so the
            # output-named operands are never read by the NEFF — they are
            # parameter padding to satisfy the hook's param-order check.
            # The kernel writes every element of yT, so the uninit result
            # buffer PJRT allocates is fine without donation.
            operands = list(args)
            if partition_name is not None:
                operands.append(bass2jax.partition_id_tensor())
            outs = bass2jax._bass_exec_p.bind(
                *operands,
                out_avals=tuple(out_avals),
                in_names=bind_in_names,
                out_names=tuple(out_names),
                lowering_input_output_aliases=(),
                sim_require_finite=True,
                sim_require_nnan=True,
                nc=nc,
            )
            return tuple(outs)

        mesh = Mesh(np.asarray(devices), ('core',))
        spec = PartitionSpec('core')
        self.sharding = NamedSharding(mesh, spec)
        self.fn = jax.jit(shard_map(
            _body, mesh=mesh,
            in_specs=(spec,) * (len(in_names) + len(out_names)),
            out_specs=(spec,) * len(out_names),
            check_rep=False))
        # On-device zero padding operands, created without host transfer;
        # never donated so they persist across calls.
        nd = self.ndev
        self.zero_pads = [
            jax.jit(lambda a=a: jnp.zeros((nd * a.shape[0],) + a.shape[1:],
                                          a.dtype), out_shardings=self.sharding)()
            for a in out_avals]
        self.const_dev = None

    def set_consts(self, const_map):
        """Upload per-core-replicated constants once; keep device-resident.
        const_map: name -> per-core np array (same for all cores)."""
        if self.dbg_name is not None:
            const_map = dict(const_map)
            const_map[self.dbg_name] = np.zeros((1, 2), np.uint32)
        dev = {}
        for name, arr in const_map.items():
            full = np.concatenate([arr] * self.ndev, axis=0)
            dev[name] = self.jax.device_put(full, self.sharding)
        for a in dev.values():
            a.block_until_ready()
        self.const_dev = dev

    def run(self, var_map, prof_tag=None):
        """var_map: name -> concatenated-over-cores np array (axis 0)."""
        t0 = time.time()
        args = []
        for name in self.in_names:
            if name in var_map:
                a = self.jax.device_put(var_map[name], self.sharding)
            else:
                a = self.const_dev[name]
            args.append(a)
        outs = self.fn(*args, *self.zero_pads)
        # fire all device->host copies concurrently; tiny tensors ride
        # along with the big one instead of paying RPC latency serially
        for o in outs:
            o.copy_to_host_async()
        t1 = time.time()
        out = {}
        ts = [t1]
        for name, o in zip(self.out_names, outs):
            out[name] = np.asarray(o)
            ts.append(time.time())
        if prof_tag is not None:
            fetch = ' '.join(f'{b-a:.3f}' for a, b in zip(ts, ts[1:]))
            print(f'[prof]     {prof_tag}: dispatch {t1-t0:.3f} fetch {fetch}')
        return out


def _get_runners(T, groups):
    key = (T, groups)
    if key not in _cache:
        import jax
        nc = _build(T)
        devs = jax.devices()[:NCORES]
        per = NCORES // groups
        _cache[key] = [_Runner(nc, devs[g*per:(g+1)*per])
                       for g in range(groups)]
    return _cache[key]


class _Results:
    exec_time_ns = None


def kernel(**inputs):
    T = int(os.environ.get('LSTM_T', T_FULL))
    groups = int(os.environ.get('LSTM_GROUPS', 2))
    prof = os.environ.get('LSTM_PROF', '0') == '1'
    import ml_dtypes
    t0 = time.time()
    runners = _get_runners(T, groups)

    # weights are cached on-device; re-upload if the caller passes
    # different weight tensors (cheap fingerprint over ~1MB)
    import hashlib
    h = hashlib.md5()
    for k in sorted(inputs):
        if k != 'x':
            h.update(np.ascontiguousarray(inputs[k]))
    wfp = h.digest()
    if getattr(runners[0], 'weight_fp', None) != wfp:
        runners[0].const_dev = None
        runners[0].weight_fp = wfp

    if runners[0].const_dev is None:
        wt, bs = {}, {}
        for L, pre in (('e0', 'enc'), ('e1', 'enc'),
                       ('d0', 'dec'), ('d1', 'dec')):
            l = L[1]
            wt[L], bs[L] = _prep_layer(
                inputs[f'{pre}_Wih{l}'], inputs[f'{pre}_Whh{l}'],
                inputs[f'{pre}_bih{l}'], inputs[f'{pre}_bhh{l}'], L != 'e0')
        bse8 = np.empty((8, 128), np.float16)
        bse8[0::2] = bs['e0']
        bse8[1::2] = bs['e1']
        ind8 = np.zeros((8, 8 * BL), np.float16)
        for i in range(8):
            ind8[i, i*BL:(i+1)*BL] = 1.0
        ind4 = np.zeros((4, 4 * BL), np.float16)
        for i in range(4):
            ind4[i, i*BL:(i+1)*BL] = 1.0
        consts = {'wt_' + L: wt[L] for L in wt}
        consts.update({'bs_' + L: bs[L] for L in bs})
        consts.update(
            bse8=bse8, ind8=ind8, ind4=ind4,
            outw=_f16(0.5 * inputs['out_W'].T),
            outb=_f16(inputs['out_b'][None, :]),
            ones=np.ones((1, BL), np.float16))
        for r in runners:
            r.set_consts(consts)
    t1 = time.time()

    x = np.asarray(inputs['x'], dtype=np.float32)[:, :T]
    y = np.empty((B, T, D), np.float32)
    BG = B // groups          # batch rows per group
    CG = NCORES // groups     # cores per group

    import concurrent.futures as cf

    import jax

    def finish(g, dx):
        # worker thread: dispatch exec, block on fetches, dequantize.
        # All the blocking happens in GIL-releasing C++ waits, so the main
        # thread keeps converting the next group's x meanwhile.
        tb = time.time()
        res = runners[g].run({'xT': dx},
                             prof_tag=f'g{g}' if prof else None)
        tc = time.time()
        # dequant: yQ [CG*BL, T*128] i8, ysc [CG*BL, 1] f32 (= rowmax/127)
        np.multiply(res['yQ'], res['ysc'], dtype=np.float32,
                    out=y[g*BG:(g+1)*BG].reshape(BG, T * 128))
        td = time.time()
        if prof:
            print(f'[prof]   g{g}: run+{tb-t1:.3f}..{tc-t1:.3f} '
                  f'dequant {td-tc:.3f} end+{td-t1:.3f}')

    # fp8 conversion is GIL-bound (ml_dtypes cast): keep it sequential in
    # the main thread, interleaved with async uploads, rather than fighting
    # over the GIL from per-group threads. One put per group: RPC overhead
    # (~0.1s/put) does not pipeline, so fewer+larger transfers win.
    jobs = []
    with cf.ThreadPoolExecutor(max(1, groups)) as ex:
        for g in range(groups):
            xg = x[g*BG:(g+1)*BG].reshape(CG, BL, T, D)
            xc = xg.transpose(0, 3, 2, 1) \
                .astype(ml_dtypes.float8_e4m3).reshape(CG * D, T * BL)
            dx = jax.device_put(xc, runners[g].sharding)
            if prof:
                print(f'[prof]   g{g}: put dispatched +{time.time()-t1:.3f}')
            jobs.append(ex.submit(finish, g, dx))
        for j in jobs:
            j.result()
    t2 = time.time()
    if prof:
        print(f'[prof] consts {t1-t0:.3f}s  pipeline {t2-t1:.3f}s')
    kernel.last_results = _Results()
    return y


# revision 39
# speedup vs baseline: 1.1313x; 1.1313x over previous
"""Trainium2 Bass kernel for nn_LSTMAutoencoder (B=512, T=256, D=H=128).

Strategy: 8-way data-parallel over batch (64/core). On-chip layout keeps
H on partitions and batch on the free dim so the recurrence needs no
transposes. Gate order is repacked host-side to [f, i, o, 2g] so one
sigmoid activation op covers all four gates (tanh(g) = 2*sigmoid(2g)-1,
recovered for free inside a fused scalar_tensor_tensor op). Encoder
layers 0/1 run as a fused wavefront (both cells share one PSUM bank,
one sigmoid op, and paired DVE ops). All weights are pre-transposed,
fp16, with biases applied via a tiny K=4/8 indicator matmul into PSUM.

Execution path: the axon tunnel moves ~40-45MB/s shared, so e2e wall
time is transfer-dominated (device exec itself is ~10ms; a full
exec+dispatch round trip is ~70ms). A custom PJRT runner (replacing
run_bass_kernel_spmd, which re-traces and re-uploads everything
including 67MB of zero-filled output buffers every call) caches the
jitted executables across calls and keeps all weight tensors
device-resident. Wire format is minimal: x goes up as fp8-e4m3
(16.8MB, t-major, upcast to fp16 on-chip; adds <2e-4 to the final
error), y comes back as int8 with a per-batch-row dynamic scale
(16.8MB; the store converts with RNE so quantization adds <=0.5
step = ~4e-3 worst case). y is built transposed on-chip (out-projection
runs as h^T @ W^T with lhsT=h) so both the device DMA and the host
dequant are contiguous. The batch is split across two 4-core meshes
whose upload/exec/download pipelines overlap via worker threads, with
the GIL-bound fp8 host conversion kept sequential on the main thread,
interleaved with async uploads. Steady state: ~0.9s/call vs 6.0s for
the baseline runner (vs ~0.78s of raw wire time for 34MB).
"""

import os
import sys
import time
import numpy as np

sys.path.insert(0, '/opt/trn_rl_repo')

B, T_FULL, D, H = 512, 256, 128, 128
NCORES = 8
BL = B // NCORES  # 64 batch per core

_cache = {}


def _f16(a):
    return np.ascontiguousarray(a).astype(np.float16)


def _prep_layer(Wih, Whh, bih, bhh, x_is_h):
    # torch gate order i,f,g,o -> [f, i, o, 2g]; transpose for lhsT use.
    # States on-chip are H2=2h, so any weight column that consumes h is
    # pre-halved (all Whh; Wih too when the layer input is a hidden state).
    def re(M):
        i, f, g, o = M[0:H], M[H:2*H], M[2*H:3*H], M[3*H:4*H]
        return np.concatenate([f, i, o, 2.0 * g], 0)
    wih = re(Wih) * (0.5 if x_is_h else 1.0)
    wt = np.concatenate([wih.T, 0.5 * re(Whh).T], 1)    # [Din, 1024]
    bs = re((bih + bhh)[:, None])[:, 0].reshape(4, H)   # [4,128]
    return _f16(wt), _f16(bs)


def _build(T):
    import concourse.bass as bass  # noqa: F401
    import concourse.tile as tile
    from concourse import bacc, mybir
    from contextlib import ExitStack

    f16, f32 = mybir.dt.float16, mybir.dt.float32
    f8 = mybir.dt.float8e4
    AO = mybir.AluOpType
    AF = mybir.ActivationFunctionType

    nc = bacc.Bacc("TRN2", target_bir_lowering=False, debug=False,
                   enable_asserts=False, num_devices=NCORES)

    def din(name, shape, dt=f16):
        return nc.dram_tensor(name, shape, dt, kind="ExternalInput").ap()

    TH = T // 2
    xTa = din('xTa', [128, TH * BL], f8)
    xTb = din('xTb', [128, (T - TH) * BL], f8)
    wts = {L: din('wt_' + L, [128, 1024]) for L in ('e0', 'e1', 'd0', 'd1')}
    bse8 = din('bse8', [8, 128])
    bss = {L: din('bs_' + L, [4, 128]) for L in ('e0', 'e1', 'd0', 'd1')}
    ind8 = din('ind8', [8, 8 * BL])
    ind4 = din('ind4', [4, 4 * BL])
    outw = din('outw', [128, 128])
    outb = din('outb', [1, 128])
    ones = din('ones', [1, BL])
    # y stored [b, t*128+d] per core as int8 with a per-row dynamic scale:
    # contiguous DMA runs on-device, transpose-free dequant on host, and
    # half the download bytes of f16. int8 convert is RNE on HW, so the
    # quantization error is <= 0.5 * rowmax/127 ~ 4e-3 of absmax worst case.
    i8 = mybir.dt.int8
    yQ = nc.dram_tensor('yQ', [BL, T * 128], i8, kind="ExternalOutput").ap()
    ysc = nc.dram_tensor('ysc', [BL, 1], f32, kind="ExternalOutput").ap()

    with tile.TileContext(nc) as tc, ExitStack() as ctx:
        cst = ctx.enter_context(tc.tile_pool(name="cst", bufs=1))
        gp = ctx.enter_context(tc.tile_pool(name="gp", bufs=4, space="PSUM"))
        yp = ctx.enter_context(tc.tile_pool(name="ypp", bufs=3, space="PSUM"))
        sb = ctx.enter_context(tc.tile_pool(name="sb", bufs=4))
        st = ctx.enter_context(tc.tile_pool(name="st", bufs=4))
        yf = ctx.enter_context(tc.tile_pool(name="yf", bufs=1))
        qp = ctx.enter_context(tc.tile_pool(name="qp", bufs=1))

        # ---- load constants into SBUF
        def cload(ap, shape, tag):
            t = cst.tile(shape, f16, tag=tag)
            nc.sync.dma_start(t[:], ap)
            return t

        xsb8 = cst.tile([128, T * BL], f8, tag='xsb8')
        nc.sync.dma_start(xsb8[:, :TH * BL], xTa)
        nc.sync.dma_start(xsb8[:, TH * BL:], xTb)
        xsb = cst.tile([128, T * BL], f16, tag='xsb')
        nc.scalar.copy(xsb[:], xsb8[:])  # one-shot fp8 -> fp16 upcast
        wsb = {L: cload(wts[L], [128, 1024], 'w' + L) for L in wts}
        bse8s = cload(bse8, [8, 128], 'bse8')
        bsbs = {L: cload(bss[L], [4, 128], 'bs' + L) for L in bss}
        ind8s = cload(ind8, [8, 8 * BL], 'ind8')
        ind4s = cload(ind4, [4, 4 * BL], 'ind4')
        outws = cload(outw, [128, 128], 'outw')
        outbs = cload(outb, [1, 128], 'outb')
        oness = cload(ones, [1, BL], 'ones')

        MM = nc.tensor.matmul
        STT = nc.vector.scalar_tensor_tensor

        # single LSTM cell: [128, BL] tiles, gates psum [128, 4*BL]
        def cell(wt, bs, x_ap, h_ap, c_ap, hout_ap, cout_ap, skip_hh, sfx):
            g = gp.tile([128, 4 * BL], f32, tag='g')
            # hh matmuls first: their input is ready one cell earlier, so
            # the PE runs them while the previous cell's elementwise tail
            # is still in flight; only ih-MMs + bias sit on the chain.
            if not skip_hh:
                for k in range(4):
                    MM(g[:, k*BL:(k+1)*BL], wt[:, 512+k*128:512+(k+1)*128],
                       h_ap, start=True, stop=False)
            for k in range(4):
                MM(g[:, k*BL:(k+1)*BL], wt[:, k*128:(k+1)*128], x_ap,
                   start=skip_hh, stop=False)
            MM(g[:, :], bs[:4, :], ind4s[:4, :], start=False, stop=True)
            s = sb.tile([128, 4 * BL], f16, tag='s')
            nc.scalar.activation(s[:], g[:], AF.Tanh, scale=0.5)
            tf, ti, to_, tg = (s[:, 0:BL], s[:, BL:2*BL],
                               s[:, 2*BL:3*BL], s[:, 3*BL:4*BL])
            u = sb.tile([128, BL], f16, tag='u')
            STT(u[:], ti, 1.0, tg, AO.add, AO.mult)       # 2*sig(i)*tanh(g)
            X = sb.tile([128, BL], f32, tag='X')
            STT(X[:], tf, 1.0, c_ap, AO.add, AO.mult)     # 2*sig(f)*C2
            STT(cout_ap, X[:], 0.5, u[:], AO.mult, AO.add)  # C2' = 2c'
            th = sb.tile([128, BL], f16, tag='th')
            nc.scalar.activation(th[:], cout_ap, AF.Tanh, scale=0.5)
            STT(hout_ap, to_, 1.0, th[:], AO.add, AO.mult)  # H2 = 2h

        # fused encoder superstep: cell0=enc0(t), cell1=enc1(t-1)
        # psum layout [128, 8*BL]: block (k, c) at (2k+c)*BL
        def fused(t, eh_prev, ec_prev, eh_new, ec_new):
            g = gp.tile([128, 8 * BL], f32, tag='g')
            x_ap = xsb[:, t*BL:(t+1)*BL]
            h0 = eh_prev[:, 0:BL]
            h1 = eh_prev[:, BL:2*BL]
            for k in range(4):
                MM(g[:, (2*k)*BL:(2*k+1)*BL],
                   wsb['e0'][:, 512+k*128:512+(k+1)*128], h0,
                   start=True, stop=False)
                MM(g[:, (2*k+1)*BL:(2*k+2)*BL],
                   wsb['e1'][:, 512+k*128:512+(k+1)*128], h1,
                   start=True, stop=False)
            for k in range(4):
                MM(g[:, (2*k)*BL:(2*k+1)*BL], wsb['e0'][:, k*128:(k+1)*128],
                   x_ap, start=False, stop=False)
                MM(g[:, (2*k+1)*BL:(2*k+2)*BL], wsb['e1'][:, k*128:(k+1)*128],
                   h0, start=False, stop=False)
            MM(g[:, :], bse8s[:8, :], ind8s[:8, :], start=False, stop=True)
            s = sb.tile([128, 8 * BL], f16, tag='s')
            nc.scalar.activation(s[:], g[:], AF.Tanh, scale=0.5)
            P = 2 * BL
            tf, ti, to_, tg = (s[:, 0:P], s[:, P:2*P],
                               s[:, 2*P:3*P], s[:, 3*P:4*P])
            u = sb.tile([128, P], f16, tag='u')
            STT(u[:], ti, 1.0, tg, AO.add, AO.mult)
            X = sb.tile([128, P], f32, tag='X')
            STT(X[:], tf, 1.0, ec_prev[:], AO.add, AO.mult)
            STT(ec_new[:], X[:], 0.5, u[:], AO.mult, AO.add)
            th = sb.tile([128, P], f16, tag='th')
            nc.scalar.activation(th[:], ec_new[:], AF.Tanh, scale=0.5)
            STT(eh_new[:], to_, 1.0, th[:], AO.add, AO.mult)

        # ---- encoder
        eh = st.tile([128, 2 * BL], f16, tag='eh')
        ec = st.tile([128, 2 * BL], f32, tag='ec')
        nc.vector.memset(eh[:], 0.0)
        nc.vector.memset(ec[:], 0.0)

        # t=0: enc0 only (h,c zero; skip hh)
        eh_n = st.tile([128, 2 * BL], f16, tag='eh')
        ec_n = st.tile([128, 2 * BL], f32, tag='ec')
        nc.vector.memset(eh_n[:], 0.0)
        nc.vector.memset(ec_n[:], 0.0)
        cell(wsb['e0'], bsbs['e0'], xsb[:, 0:BL], None, ec[:, 0:BL],
             eh_n[:, 0:BL], ec_n[:, 0:BL], True, 'e0z')
        eh, ec = eh_n, ec_n

        for t in range(1, T):
            eh_n = st.tile([128, 2 * BL], f16, tag='eh')
            ec_n = st.tile([128, 2 * BL], f32, tag='ec')
            fused(t, eh, ec, eh_n, ec_n)
            eh, ec = eh_n, ec_n

        # tail: enc1 consumes h0(T-1)
        h1f = st.tile([128, BL], f16, tag='h1f')
        c1f = st.tile([128, BL], f32, tag='c1f')
        cell(wsb['e1'], bsbs['e1'], eh[:, 0:BL], eh[:, BL:2*BL],
             ec[:, BL:2*BL], h1f[:], c1f[:], False, 'e1z')

        # ---- decoder
        hx = h1f
        hd0 = st.tile([128, BL], f16, tag='hd0')
        cd0 = st.tile([128, BL], f32, tag='cd0')
        hd1 = st.tile([128, BL], f16, tag='hd1')
        cd1 = st.tile([128, BL], f32, tag='cd1')
        for z in (hd0, cd0, hd1, cd1):
            nc.vector.memset(z[:], 0.0)

        ysb = yf.tile([BL, T * 128], f16, tag='ysb')
        for t in range(T):
            hd0n = st.tile([128, BL], f16, tag='hd0')
            cd0n = st.tile([128, BL], f32, tag='cd0')
            cell(wsb['d0'], bsbs['d0'], hx[:], hd0[:], cd0[:],
                 hd0n[:], cd0n[:], t == 0, 'd0')
            hd1n = st.tile([128, BL], f16, tag='hd1')
            cd1n = st.tile([128, BL], f32, tag='cd1')
            cell(wsb['d1'], bsbs['d1'], hd0n[:], hd1[:], cd1[:],
                 hd1n[:], cd1n[:], t == 0, 'd1')
            hd0, cd0, hd1, cd1 = hd0n, cd0n, hd1n, cd1n
            # transposed out-projection: y^T[b,d] = (h^T @ W_out^T)[b,d]
            # (lhsT = hd1 directly; outws already holds 0.5*W_out^T)
            y = yp.tile([BL, 128], f32, tag='yp')
            MM(y[:], hd1[:], outws[:], start=True, stop=False)
            MM(y[:], oness[:1, :], outbs[:1, :], start=False, stop=True)
            nc.scalar.copy(ysb[:, t*128:(t+1)*128], y[:])
            hx = hd1

        # ---- dynamic per-row int8 quantization of y
        red = sb.tile([BL, 1], f32, tag='red')
        nc.vector.tensor_reduce(red[:], ysb[:], mybir.AxisListType.X,
                                AO.max, apply_absolute_value=True)
        ysch = sb.tile([BL, 1], f32, tag='ysch')
        nc.vector.tensor_scalar_mul(ysch[:], red[:], 1.0 / 127.0)
        nc.sync.dma_start(ysc, ysch[:])
        sc = sb.tile([BL, 1], f32, tag='sc')
        nc.vector.reciprocal(sc[:], ysch[:])
        q = qp.tile([BL, T * 128], i8, tag='q')
        nc.scalar.activation(q[:], ysb[:], AF.Copy, scale=sc[:])
        nc.sync.dma_start(yQ, q[:])

    nc.compile()
    return nc


class _Runner:
    """Cached PJRT execution of a compiled Bass module on a group of axon
    cores.

    Replaces run_bass_kernel_spmd's per-call closure (which re-traces,
    re-uploads every input, and uploads zero-filled output buffers each
    call). Constants live on device; only x moves per call. Running two
    runners on disjoint 4-core meshes from two threads pipelines the
    upload/exec/download of one batch half against the other.
    """

    def __init__(self, nc, devices):
        import jax
        import jax.numpy as jnp
        from jax.sharding import Mesh, PartitionSpec, NamedSharding
        from jax.experimental.shard_map import shard_map
        from concourse import bass2jax, mybir

        bass2jax.install_neuronx_cc_hook()
        self.jax = jax
        self.nc = nc
        self.ndev = len(devices)

        partition_name = (nc.partition_id_tensor.name
                          if nc.partition_id_tensor else None)
        dbg_name = nc.dbg_addr.name if nc.dbg_addr is not None else None
        if nc.dbg_addr is not None and nc.dbg_callbacks:
            raise RuntimeError('dbg_callbacks unsupported on axon client')

        in_names, out_names, out_avals = [], [], []
        for alloc in nc.m.functions[0].allocations:
            if not isinstance(alloc, mybir.MemoryLocationSet):
                continue
            name = alloc.memorylocations[0].name
            if alloc.kind == 'ExternalInput':
                if name != partition_name:
                    in_names.append(name)
            elif alloc.kind == 'ExternalOutput':
                out_names.append(name)
                out_avals.append(jax.core.ShapedArray(
                    tuple(alloc.tensor_shape), mybir.dt.np(alloc.dtype)))
        self.in_names = in_names
        self.out_names = out_names
        self.dbg_name = dbg_name

        bind_in_names = tuple(in_names) + tuple(out_names)
        if partition_name is not None:
            bind_in_names += (partition_name,)

        def _body(*args):
            # args = real inputs + placeholder output operands. The hook
            # renames each NEFF output tensor to output{i} only, so the
            # output-named operands are never read by the NEFF — they are
            # parameter padding to satisfy the hook's param-order check.
            # The kernel writes every element of yT, so the uninit result
            # buffer PJRT allocates is fine without donation.
            operands = list(args)
            if partition_name is not None:
                operands.append(bass2jax.partition_id_tensor())
            outs = bass2jax._bass_exec_p.bind(
                *operands,
                out_avals=tuple(out_avals),
                in_names=bind_in_names,
                out_names=tuple(out_names),
                lowering_input_output_aliases=(),
                sim_require_finite=True,
                sim_require_nnan=True,
                nc=nc,
            )
            return tuple(outs)

        mesh = Mesh(np.asarray(devices), ('core',))
        spec = PartitionSpec('core')
        self.sharding = NamedSharding(mesh, spec)
        self.fn = jax.jit(shard_map(
            _body, mesh=mesh,
            in_specs=(spec,) * (len(in_names) + len(out_names)),
            out_specs=(spec,) * len(out_names),
            check_rep=False))
        # On-device zero padding operands, created without host transfer;
        # never donated so they persist across calls.
        nd = self.ndev
        self.zero_pads = [
            jax.jit(lambda a=a: jnp.zeros((nd * a.shape[0],) + a.shape[1:],
                                          a.dtype), out_shardings=self.sharding)()
            for a in out_avals]
        self.const_dev = None

    def set_consts(self, const_map):
        """Upload per-core-replicated constants once; keep device-resident.
        const_map: name -> per-core np array (same for all cores)."""
        if self.dbg_name is not None:
            const_map = dict(const_map)
            const_map[self.dbg_name] = np.zeros((1, 2), np.uint32)
        dev = {}
        for name, arr in const_map.items():
            full = np.concatenate([arr] * self.ndev, axis=0)
            dev[name] = self.jax.device_put(full, self.sharding)
        for a in dev.values():
            a.block_until_ready()
        self.const_dev = dev

    def run(self, var_map, prof_tag=None):
        """var_map: name -> concatenated-over-cores np array (axis 0)."""
        t0 = time.time()
        args = []
        for name in self.in_names:
            if name in var_map:
                a = self.jax.device_put(var_map[name], self.sharding)
            else:
                a = self.const_dev[name]
            args.append(a)
        outs = self.fn(*args, *self.zero_pads)
        # fire all device->host copies concurrently; tiny tensors ride
        # along with the big one instead of paying RPC latency serially
        for o in outs:
            o.copy_to_host_async()
        t1 = time.time()
        out = {}
        ts = [t1]
        for name, o in zip(self.out_names, outs):
            out[name] = np.asarray(o)
            ts.append(time.time())
        if prof_tag is not None:
            fetch = ' '.join(f'{b-a:.3f}' for a, b in zip(ts, ts[1:]))
            print(f'[prof]     {prof_tag}: dispatch {t1-t0:.3f} fetch {fetch}')
        return out


def _get_runners(T, groups):
    key = (T, groups)
    if key not in _cache:
        import jax
        nc = _build(T)
        devs = jax.devices()[:NCORES]
        per = NCORES // groups
        _cache[key] = [_Runner(nc, devs[g*per:(g+1)*per])
                       for g in range(groups)]
    return _cache[key]


class _Results:
    exec_time_ns = None


def kernel(**inputs):
    T = int(os.environ.get('LSTM_T', T_FULL))
    groups = int(os.environ.get('LSTM_GROUPS', 2))
    prof = os.environ.get('LSTM_PROF', '0') == '1'
    import ml_dtypes
    t0 = time.time()
    runners = _get_runners(T, groups)

    # weights are cached on-device; re-upload if the caller passes
    # different weight tensors (cheap fingerprint over ~1MB)
    import hashlib
    h = hashlib.md5()
    for k in sorted(inputs):
        if k != 'x':
            h.update(np.ascontiguousarray(inputs[k]))
    wfp = h.digest()
    if getattr(runners[0], 'weight_fp', None) != wfp:
        runners[0].const_dev = None
        runners[0].weight_fp = wfp

    if runners[0].const_dev is None:
        wt, bs = {}, {}
        for L, pre in (('e0', 'enc'), ('e1', 'enc'),
                       ('d0', 'dec'), ('d1', 'dec')):
            l = L[1]
            wt[L], bs[L] = _prep_layer(
                inputs[f'{pre}_Wih{l}'], inputs[f'{pre}_Whh{l}'],
                inputs[f'{pre}_bih{l}'], inputs[f'{pre}_bhh{l}'], L != 'e0')
        bse8 = np.empty((8, 128), np.float16)
        bse8[0::2] = bs['e0']
        bse8[1::2] = bs['e1']
        ind8 = np.zeros((8, 8 * BL), np.float16)
        for i in range(8):
            ind8[i, i*BL:(i+1)*BL] = 1.0
        ind4 = np.zeros((4, 4 * BL), np.float16)
        for i in range(4):
            ind4[i, i*BL:(i+1)*BL] = 1.0
        consts = {'wt_' + L: wt[L] for L in wt}
        consts.update({'bs_' + L: bs[L] for L in bs})
        consts.update(
            bse8=bse8, ind8=ind8, ind4=ind4,
            outw=_f16(0.5 * inputs['out_W'].T),
            outb=_f16(inputs['out_b'][None, :]),
            ones=np.ones((1, BL), np.float16))
        for r in runners:
            r.set_consts(consts)
    t1 = time.time()

    x = np.asarray(inputs['x'], dtype=np.float32)[:, :T]
    y = np.empty((B, T, D), np.float32)
    BG = B // groups          # batch rows per group
    CG = NCORES // groups     # cores per group

    import concurrent.futures as cf

    import jax
    TH = T // 2

    def finish(g, da, db):
        # worker thread: dispatch exec, block on fetches, dequantize.
        # All the blocking happens in GIL-releasing C++ waits, so the main
        # thread keeps converting the next group's x meanwhile.
        tb = time.time()
        res = runners[g].run({'xTa': da, 'xTb': db},
                             prof_tag=f'g{g}' if prof else None)
        tc = time.time()
        # dequant: yQ [CG*BL, T*128] i8, ysc [CG*BL, 1] f32 (= rowmax/127)
        np.multiply(res['yQ'], res['ysc'], dtype=np.float32,
                    out=y[g*BG:(g+1)*BG].reshape(BG, T * 128))
        td = time.time()
        if prof:
            print(f'[prof]   g{g}: run+{tb-t1:.3f}..{tc-t1:.3f} '
                  f'dequant {td-tc:.3f} end+{td-t1:.3f}')

    # fp8 conversion is GIL-bound (ml_dtypes cast): keep it sequential in
    # the main thread, interleaved with async uploads, rather than fighting
    # over the GIL from per-group threads.
    jobs = []
    with cf.ThreadPoolExecutor(max(1, groups)) as ex:
        for g in range(groups):
            xg = x[g*BG:(g+1)*BG].reshape(CG, BL, T, D)
            xa = xg[:, :, :TH].transpose(0, 3, 2, 1) \
                .astype(ml_dtypes.float8_e4m3).reshape(CG * D, TH * BL)
            da = jax.device_put(xa, runners[g].sharding)
            xb = xg[:, :, TH:].transpose(0, 3, 2, 1) \
                .astype(ml_dtypes.float8_e4m3).reshape(CG * D, (T - TH) * BL)
            db = jax.device_put(xb, runners[g].sharding)
            if prof:
                print(f'[prof]   g{g}: put dispatched +{time.time()-t1:.3f}')
            jobs.append(ex.submit(finish, g, da, db))
        for j in jobs:
            j.result()
    t2 = time.time()
    if prof:
        print(f'[prof] consts {t1-t0:.3f}s  pipeline {t2-t1:.3f}s')
    kernel.last_results = _Results()
    return y
